# revision 1
# baseline (speedup 1.0000x reference)
"""HeatmapMSELoss Trainium2 kernel.

Computes mean((heatmaps_pred - heatmaps_gt)^2) where heatmaps_gt is an
isotropic 2D gaussian (sigma=1, peak 1) rendered at the projection of each
3D joint into each view.

Key identity: the gaussian separates, gt[h,w] = gy[h] * gx[w], so

  sum_hw (pred - gt)^2 = sum_hw pred^2 - 2 * gy^T (pred @ gx) + (sum gy^2)(sum gx^2)

The 142MB gt tensor is never materialized. Per (b,v,j) slice the device
computes sum(pred^2) (scalar-engine square + accumulate) and
m' = pred^T @ gy (one matmul, PSUM column), then a fused DVE
multiply+reduce against gx. The tiny 1D gaussians (2.2MB total) and the
final scalar combine are done on host in float64.

Sharding: data-parallel over batch, 4 batches per core across 8 cores.
"""

import numpy as np

import concourse.bacc as bacc
import concourse.bass as bass
import concourse.tile as tile
from concourse import mybir
from concourse.bass_utils import run_bass_kernel_spmd

B, V, J, H, W = 32, 4, 17, 128, 128
N_CORES = 8
B_LOC = B // N_CORES          # 4 batches per core
GROUPS = B_LOC * V            # 16 (b,v) groups per core
SLICES = GROUPS * J           # 272 slices per core

_CACHE = {}


GPB = 2                    # (b,v) groups per block
NBLK = GROUPS // GPB       # blocks per core
JB = GPB * J               # joints (slices) per block

# chunk sizes (in slices) over the 272 per-core slices: small chunks at the
# start (fast pipeline ramp: compute starts after a ~1us DMA, not ~3us) and
# at the end (short tail after the last DMA lands)
CHUNKS = [4, 4, 4, 5] + [17] * 14 + [9, 8]
assert sum(CHUNKS) == SLICES


def _build_nc(passes=1, chunks=None, load_bufs=6):
    # Bacc (not raw Bass): its finalize() runs the legalization passes that
    # split multi-wait instructions (matmul can carry at most 1 sync wait).
    nc = bacc.Bacc()
    f32 = mybir.dt.float32
    chunks = list(CHUNKS) if chunks is None else list(chunks)
    nck = len(chunks)
    maxck = max(chunks)

    pred = nc.declare_dram_parameter("pred", [SLICES, H, W], f32, isOutput=False)
    gyt = nc.declare_dram_parameter("gyt", [H, SLICES], f32, isOutput=False)
    gxt = nc.declare_dram_parameter("gxt", [W, SLICES], f32, isOutput=False)
    partials = nc.declare_dram_parameter("partials", [128, 2, nck], f32, isOutput=True)

    with tile.TileContext(nc) as tc:
        with (
            tc.tile_pool(name="consts", bufs=1) as consts,
            tc.tile_pool(name="loads", bufs=load_bufs) as loads,
            tc.tile_pool(name="sq", bufs=2) as sqpool,
            tc.tile_pool(name="prod", bufs=2) as prodpool,
            tc.tile_pool(name="psum", bufs=4, space="PSUM") as psumpool,
            tc.tile_pool(name="outs", bufs=1) as outs,
        ):
            # warm-up ACT so the Square table-set load (~2.7us) overlaps the
            # first pred DMA instead of stalling the first real ACT
            warm = consts.tile([128, 1], f32)
            nc.vector.memset(warm[:], 0.0)
            wsq = consts.tile([128, 1], f32)
            nc.scalar.activation(
                out=wsq[:], in_=warm[:], func=mybir.ActivationFunctionType.Square
            )

            gyt_t = consts.tile([H, SLICES], f32)
            nc.sync.dma_start(out=gyt_t[:], in_=gyt[:, :])
            gxt_t = consts.tile([W, SLICES], f32)
            nc.sync.dma_start(out=gxt_t[:], in_=gxt[:, :])

            outcols = outs.tile([128, 2, nck], f32)

            for _p in range(passes):
                s0 = 0
                for c, csz in enumerate(chunks):
                    t = loads.tile([H, maxck, W], f32, tag="loads")
                    nc.sync.dma_start(
                        out=t[:, :csz, :],
                        in_=pred[s0 : s0 + csz].rearrange("s h w -> h s w"),
                    )

                    # s1: per-partition sum of pred^2 over (s, w)
                    sq = sqpool.tile([H, maxck, W], f32, tag="sq")
                    nc.scalar.activation(
                        out=sq[:, :csz, :],
                        in_=t[:, :csz, :],
                        func=mybir.ActivationFunctionType.Square,
                        accum_out=outcols[:, 0, c : c + 1],
                    )

                    # s2: m'_s = pred_s^T @ gy_s per slice -> psum column
                    ps = psumpool.tile([128, maxck], f32, tag="psum")
                    for sj in range(csz):
                        s = s0 + sj
                        nc.tensor.matmul(
                            ps[:, sj : sj + 1],
                            t[:, sj, :],
                            gyt_t[:, s : s + 1],
                            start=True,
                            stop=True,
                        )
                    # dot with gx, then per-partition sum over slices
                    prod = prodpool.tile([128, maxck], f32, tag="prod")
                    nc.vector.tensor_mul(
                        prod[:, :csz], ps[:, :csz], gxt_t[:, s0 : s0 + csz]
                    )
                    nc.vector.reduce_sum(
                        outcols[:, 1, c : c + 1], prod[:, :csz],
                        axis=mybir.AxisListType.X,
                    )
                    s0 += csz

            nc.sync.dma_start(out=partials[:, :, :], in_=outcols[:])

    nc.finalize()  # Bacc: runs legalization (wait splitting) + regalloc
    return nc


def _gaussians(proj_mats_batch, joints_3d_gt_batch):
    """1D gaussians gy [B,V,J,H], gx [B,V,J,W] in float32 (reference math)."""
    joints = joints_3d_gt_batch.astype(np.float32)
    ones = np.ones(joints.shape[:-1] + (1,), dtype=np.float32)
    joints_h = np.concatenate([joints, ones], axis=-1)  # [B, J, 4]
    proj = np.einsum(
        "bvcd,bjd->bvjc", proj_mats_batch.astype(np.float32), joints_h
    ).astype(np.float32)  # [B, V, J, 3]
    joints_2d = proj[..., :2] / proj[..., 2:3]  # (x, y)
    xs = np.arange(W, dtype=np.float32)
    ys = np.arange(H, dtype=np.float32)
    dx2 = (xs - joints_2d[..., 0, None]) ** 2  # [B,V,J,W]
    dy2 = (ys - joints_2d[..., 1, None]) ** 2  # [B,V,J,H]
    gx = np.exp(-0.5 * dx2).astype(np.float32)
    gy = np.exp(-0.5 * dy2).astype(np.float32)
    return gy, gx


def kernel(heatmaps_pred, proj_mats_batch, joints_3d_gt_batch, joints_3d_valid_batch,
           _profile=None):
    heatmaps_pred = np.ascontiguousarray(np.asarray(heatmaps_pred, dtype=np.float32))
    gy, gx = _gaussians(np.asarray(proj_mats_batch), np.asarray(joints_3d_gt_batch))

    # s3 = sum over slices of (sum_h gy^2) * (sum_w gx^2), exact in f64
    s3 = float(
        ((gy.astype(np.float64) ** 2).sum(-1) * (gx.astype(np.float64) ** 2).sum(-1)).sum()
    )

    if "nc" not in _CACHE:
        _CACHE["nc"] = _build_nc()
    nc = _CACHE["nc"]

    in_maps = []
    for c in range(N_CORES):
        bsl = slice(B_LOC * c, B_LOC * (c + 1))
        # slice order: (b_local, v, j) -> s ; tiles are [H|W, SLICES]
        gyt = np.ascontiguousarray(gy[bsl].reshape(SLICES, H).T)
        gxt = np.ascontiguousarray(gx[bsl].reshape(SLICES, W).T)
        in_maps.append(
            {
                "pred": heatmaps_pred[bsl].reshape(SLICES, H, W),
                "gyt": gyt,
                "gxt": gxt,
            }
        )

    res = run_bass_kernel_spmd(nc, in_maps, core_ids=list(range(N_CORES)))
    if _profile is not None:
        _profile["result"] = res
        _profile["in_maps"] = in_maps

    s1 = 0.0
    s2 = 0.0
    for c in range(N_CORES):
        p = res.results[c]["partials"].astype(np.float64)
        s1 += p[:, 0, :].sum()
        s2 += p[:, 1, :].sum()

    total = s1 - 2.0 * s2 + s3
    return np.float32(total / (B * V * J * H * W))



# revision 2
# speedup vs baseline: 1.6580x; 1.6580x over previous
"""HeatmapMSELoss Trainium2 kernel (bf16 streaming version).

Computes mean((heatmaps_pred - heatmaps_gt)^2) where heatmaps_gt is an
isotropic 2D gaussian (sigma=1, peak 1) rendered at the projection of each
3D joint into each view.

Key identity: the gaussian separates, gt[h,w] = gy[h] * gx[w], so

  sum_hw (pred - gt)^2 = sum_hw pred^2 - 2 * gy^T (pred @ gx) + (sum gy^2)(sum gx^2)

The 142MB gt tensor is never materialized. The dominant cost is streaming
pred through the core once; pred is cast to bf16 on host (error on the
final scalar ~1e-5, far inside tolerance) and pre-transposed to h-major
[H, SLICES, W] so each DMA descriptor moves csz*W*2 >= 512 contiguous
bytes -- halving HBM traffic vs f32 without the sub-512B descriptor
penalty.

Per chunk of csz slices the square-sum work is split between the ACT
engine (activation Square + accumulator, 0.83ns/el) and the DVE engine
(tensor_mul square at 2x bf16 rate + halving tree of tensor_adds + one
short reduce), because ACT alone cannot keep up with the bf16 DMA stream.
The cross term uses one per-slice matmul (pred_s^T @ gy_s, bf16) into
PSUM, then a fused DVE multiply by gx and reduce.

Sharding: data-parallel over batch, 4 batches per core across 8 cores;
the tiny per-chunk column partials are combined on host in float64.
"""

import numpy as np
import ml_dtypes

import concourse.bacc as bacc
import concourse.bass as bass
import concourse.tile as tile
from concourse import mybir
from concourse.bass_utils import run_bass_kernel_spmd

B, V, J, H, W = 32, 4, 17, 128, 128
N_CORES = 8
B_LOC = B // N_CORES          # 4 batches per core
SLICES = B_LOC * V * J        # 272 slices per core

_CACHE = {}

# (chunk_size, act_share) in slices; dve_share = chunk_size - act_share.
# Middle chunks are large (amortize per-instruction overheads), tail chunks
# shrink so the last chunk's compute is short after its DMA lands.
CHUNKS = [
    (10, 6), (24, 15),
    (34, 21), (34, 21), (34, 21), (34, 21), (34, 21), (34, 21),
    (20, 12), (10, 6), (4, 2),
]
assert sum(c for c, _ in CHUNKS) == SLICES
NCK = len(CHUNKS)
MAXCK = max(c for c, _ in CHUNKS)
MAXA = max(a for _, a in CHUNKS)
MAXD = max(c - a for c, a in CHUNKS)


def _build_nc():
    nc = bacc.Bacc()
    f32 = mybir.dt.float32
    bf16 = mybir.dt.bfloat16

    pred = nc.declare_dram_parameter("pred", [H, SLICES, W], bf16, isOutput=False)
    gyt = nc.declare_dram_parameter("gyt", [H, SLICES], bf16, isOutput=False)
    gxt = nc.declare_dram_parameter("gxt", [W, SLICES], f32, isOutput=False)
    partials = nc.declare_dram_parameter("partials", [128, 3, NCK], f32, isOutput=True)

    with tile.TileContext(nc) as tc:
        with (
            tc.tile_pool(name="consts", bufs=1) as consts,
            tc.tile_pool(name="loads", bufs=6) as loads,
            tc.tile_pool(name="sq", bufs=1) as sqpool,
            tc.tile_pool(name="prod", bufs=2) as prodpool,
            tc.tile_pool(name="psum", bufs=4, space="PSUM") as psumpool,
            tc.tile_pool(name="outs", bufs=1) as outs,
        ):
            # warm-up ACT so the Square table-set load (~1.3us) overlaps the
            # first pred DMA instead of stalling the first real ACT
            warm = consts.tile([128, 1], f32)
            nc.vector.memset(warm[:], 0.0)
            wsq = consts.tile([128, 1], f32)
            nc.scalar.activation(
                out=wsq[:], in_=warm[:], func=mybir.ActivationFunctionType.Square
            )

            gyt_t = consts.tile([H, SLICES], bf16)
            gxt_t = consts.tile([W, SLICES], f32)
            actout = consts.tile([128, MAXA * W], bf16)
            outcols = outs.tile([128, 3, NCK], f32)

            s0 = 0
            for c, (csz, ak) in enumerate(CHUNKS):
                dk = csz - ak
                t = loads.tile([128, MAXCK * W], bf16, tag="loads")
                nc.sync.dma_start(
                    out=t[:, : csz * W],
                    in_=pred[:, s0 : s0 + csz, :].rearrange("h s w -> h (s w)"),
                )
                if c == 0:
                    # issue the small gaussian loads after the first pred
                    # chunk so the pred stream starts as early as possible
                    nc.sync.dma_start(out=gyt_t[:], in_=gyt[:, :])
                    nc.sync.dma_start(out=gxt_t[:], in_=gxt[:, :])

                # ACT share: sum of pred^2 over its slices -> outcols row 0
                nc.scalar.activation(
                    out=actout[:, : ak * W],
                    in_=t[:, : ak * W],
                    func=mybir.ActivationFunctionType.Square,
                    accum_out=outcols[:, 0, c : c + 1],
                )

                # DVE share: square at 2x bf16 rate, halving tree, short
                # reduce -> outcols row 1
                sq2 = sqpool.tile([128, MAXD * W], bf16, tag="sq")
                n = dk * W
                nc.vector.tensor_mul(
                    sq2[:, :n], t[:, ak * W : csz * W], t[:, ak * W : csz * W]
                )
                while n > 256:
                    h = n // 2
                    nc.vector.tensor_add(sq2[:, :h], sq2[:, :h], sq2[:, h:n])
                    n = h
                nc.vector.reduce_sum(
                    outcols[:, 1, c : c + 1], sq2[:, :n], axis=mybir.AxisListType.X
                )

                # cross term: m'_s = pred_s^T @ gy_s per slice -> psum column
                ps = psumpool.tile([128, MAXCK], f32, tag="psum")
                for sj in range(csz):
                    nc.tensor.matmul(
                        ps[:, sj : sj + 1],
                        t[:, sj * W : (sj + 1) * W],
                        gyt_t[:, s0 + sj : s0 + sj + 1],
                        start=True,
                        stop=True,
                    )
                prod = prodpool.tile([128, MAXCK], f32, tag="prod")
                nc.vector.tensor_mul(
                    prod[:, :csz], ps[:, :csz], gxt_t[:, s0 : s0 + csz]
                )
                nc.vector.reduce_sum(
                    outcols[:, 2, c : c + 1], prod[:, :csz],
                    axis=mybir.AxisListType.X,
                )
                s0 += csz

            nc.sync.dma_start(out=partials[:, :, :], in_=outcols[:])

    nc.finalize()
    return nc


def _gaussians(proj_mats_batch, joints_3d_gt_batch):
    """1D gaussians gy [B,V,J,H], gx [B,V,J,W] in float32 (reference math)."""
    joints = joints_3d_gt_batch.astype(np.float32)
    ones = np.ones(joints.shape[:-1] + (1,), dtype=np.float32)
    joints_h = np.concatenate([joints, ones], axis=-1)  # [B, J, 4]
    proj = np.einsum(
        "bvcd,bjd->bvjc", proj_mats_batch.astype(np.float32), joints_h
    ).astype(np.float32)  # [B, V, J, 3]
    joints_2d = proj[..., :2] / proj[..., 2:3]  # (x, y)
    xs = np.arange(W, dtype=np.float32)
    ys = np.arange(H, dtype=np.float32)
    dx2 = (xs - joints_2d[..., 0, None]) ** 2  # [B,V,J,W]
    dy2 = (ys - joints_2d[..., 1, None]) ** 2  # [B,V,J,H]
    gx = np.exp(-0.5 * dx2).astype(np.float32)
    gy = np.exp(-0.5 * dy2).astype(np.float32)
    return gy, gx


def kernel(heatmaps_pred, proj_mats_batch, joints_3d_gt_batch, joints_3d_valid_batch,
           _profile=None):
    heatmaps_pred = np.asarray(heatmaps_pred, dtype=np.float32)
    gy, gx = _gaussians(np.asarray(proj_mats_batch), np.asarray(joints_3d_gt_batch))

    # s3 = sum over slices of (sum_h gy^2) * (sum_w gx^2), exact in f64
    s3 = float(
        ((gy.astype(np.float64) ** 2).sum(-1) * (gx.astype(np.float64) ** 2).sum(-1)).sum()
    )

    if "nc" not in _CACHE:
        _CACHE["nc"] = _build_nc()
    nc = _CACHE["nc"]

    pred_bf = heatmaps_pred.astype(ml_dtypes.bfloat16)
    in_maps = []
    for c in range(N_CORES):
        bsl = slice(B_LOC * c, B_LOC * (c + 1))
        # slice order: (b_local, v, j) -> s ; pred h-major [H, SLICES, W]
        pred_t = np.ascontiguousarray(
            pred_bf[bsl].reshape(SLICES, H, W).transpose(1, 0, 2)
        )
        gyt = np.ascontiguousarray(
            gy[bsl].reshape(SLICES, H).T.astype(ml_dtypes.bfloat16)
        )
        gxt = np.ascontiguousarray(gx[bsl].reshape(SLICES, W).T)
        in_maps.append({"pred": pred_t, "gyt": gyt, "gxt": gxt})

    res = run_bass_kernel_spmd(nc, in_maps, core_ids=list(range(N_CORES)))
    if _profile is not None:
        _profile["result"] = res
        _profile["in_maps"] = in_maps

    s1 = 0.0
    s2 = 0.0
    for c in range(N_CORES):
        p = res.results[c]["partials"].astype(np.float64)
        s1 += p[:, 0, :].sum() + p[:, 1, :].sum()
        s2 += p[:, 2, :].sum()

    total = s1 - 2.0 * s2 + s3
    return np.float32(total / (B * V * J * H * W))


# revision 4
# speedup vs baseline: 1.8847x; 1.1368x over previous
"""HeatmapMSELoss Trainium2 kernel (mixed fp8/bf16, 3-engine version).

Computes mean((heatmaps_pred - heatmaps_gt)^2) where heatmaps_gt is an
isotropic 2D gaussian (sigma=1, peak 1) rendered at the projection of each
3D joint into each view.

Key identity: the gaussian separates, gt[h,w] = gy[h] * gx[w], so

  sum_hw (pred - gt)^2 = sum_hw pred^2 - 2 * gy^T (pred @ gx) + (sum gy^2)(sum gx^2)

The gt tensor is never materialized. pred is pre-transposed on host to
h-major [H, S, W] (so every DMA descriptor is >= 512 contiguous bytes)
and split into two streams: an fp8e4m3 stream whose squares are summed by
the ACT engine (activation Square + accumulator) and the Pool engine
(gpsimd multiply + halving adds into a persistent f32 accumulator), and a
bf16 stream whose squares are summed by the DVE engine (tensor_mul at 2x
bf16 rate + halving tree + short reduce). This keeps all three
element-wise engines busy in parallel, which is what bounds the kernel
(the quantized streams need only ~17us of DMA).

fp8 quantization biases sum(pred^2) by ~+5e-4 relative (mean squared
rounding error); harness tolerance is 2e-2.

The cross term uses one per-slice matmul (pred_s^T @ gy_s) into a
persistent PSUM tile; the multiply by gx and reduction run as two grouped
DVE ops over hundreds of columns at once.

Sharding: data-parallel over batch, 4 batches per core across 8 cores;
per-chunk column partials are combined on host in float64.
"""

import numpy as np
import ml_dtypes

import concourse.bacc as bacc
import concourse.bass as bass
import concourse.tile as tile
from concourse import mybir
from concourse.bass_utils import run_bass_kernel_spmd

B, V, J, H, W = 32, 4, 17, 128, 128
N_CORES = 8
B_LOC = B // N_CORES          # 4 batches per core
SLICES = B_LOC * V * J        # 272 slices per core

_CACHE = {}

# Per round: (act_slices, dve_slices, pool_slices).
# ACT+Pool slices stream as fp8, DVE slices as bf16. Round sizes decrease
# toward the end so the last round's compute is short after its DMA lands.
ROUNDS = [
    (10, 8, 4),
    (30, 20, 4),
    (29, 19, 4),
    (27, 17, 4),
    (24, 16, 4),
    (17, 11, 4),
    (10, 8, 2),
]
assert sum(a + d + p for a, d, p in ROUNDS) == SLICES
NR = len(ROUNDS)
MAXA = max(a for a, _, _ in ROUNDS)
MAXD = max(d for _, d, _ in ROUNDS)
MAXP = max(p for _, _, p in ROUNDS)
S8 = sum(a + p for a, _, p in ROUNDS)   # fp8 slices total
S16 = sum(d for _, d, _ in ROUNDS)      # bf16 slices total

# outcols layout: [ACT accums (NR) | DVE reduces (NR) | poolacc reduce (1) |
#                  prodA (1) | prodB (1)]
NC = 2 * NR + 3


def _build_nc():
    nc = bacc.Bacc()
    f32 = mybir.dt.float32
    bf16 = mybir.dt.bfloat16
    fp8 = mybir.dt.float8e4

    pred8 = nc.declare_dram_parameter("pred8", [H, S8, W], fp8, isOutput=False)
    pred16 = nc.declare_dram_parameter("pred16", [H, S16, W], bf16, isOutput=False)
    gy8 = nc.declare_dram_parameter("gy8", [H, S8], fp8, isOutput=False)
    gy16 = nc.declare_dram_parameter("gy16", [H, S16], bf16, isOutput=False)
    gx = nc.declare_dram_parameter("gx", [W, SLICES], bf16, isOutput=False)
    partials = nc.declare_dram_parameter("partials", [128, NC], f32, isOutput=True)

    # columns in the persistent PSUM m' tiles follow global slice order:
    # per round, fp8 slices (ACT then Pool shares) first, then bf16 slices.
    n_last = sum(ROUNDS[-1])
    n_main = SLICES - n_last

    with tile.TileContext(nc) as tc:
        with (
            tc.tile_pool(name="consts", bufs=1) as consts,
            tc.tile_pool(name="l8", bufs=3) as l8pool,
            tc.tile_pool(name="l16", bufs=3) as l16pool,
            tc.tile_pool(name="work", bufs=1) as work,
            tc.tile_pool(name="psum", bufs=1, space="PSUM") as psumpool,
            tc.tile_pool(name="outs", bufs=1) as outs,
        ):
            # warm-up ACT so the Square table-set load overlaps the first DMA
            warm = consts.tile([128, 1], f32)
            nc.vector.memset(warm[:], 0.0)
            wsq = consts.tile([128, 1], f32)
            nc.scalar.activation(
                out=wsq[:], in_=warm[:], func=mybir.ActivationFunctionType.Square
            )

            gy8_t = consts.tile([H, S8], fp8)
            gy16_t = consts.tile([H, S16], bf16)
            gx_t = consts.tile([W, SLICES], bf16)
            actout = consts.tile([128, MAXA * W], bf16)
            sq16 = consts.tile([128, MAXD * W], bf16)
            sq8 = consts.tile([128, MAXP * W], bf16)
            poolacc = consts.tile([128, 256], f32)
            nc.gpsimd.memset(poolacc[:], 0.0)
            outcols = outs.tile([128, NC], f32)

            psA = psumpool.tile([128, n_main], f32, tag="psA")
            psB = psumpool.tile([128, n_last], f32, tag="psB")

            s8_0 = 0   # running fp8 slice offset
            s16_0 = 0  # running bf16 slice offset
            g0 = 0     # running global slice offset (psum/gx column order)
            for r, (ak, dk, pk) in enumerate(ROUNDS):
                fk = ak + pk  # fp8 slices this round
                t16 = l16pool.tile([128, MAXD * W], bf16, tag="l16")
                nc.sync.dma_start(
                    out=t16[:, : dk * W],
                    in_=pred16[:, s16_0 : s16_0 + dk, :].rearrange("h s w -> h (s w)"),
                )
                t8 = l8pool.tile([128, (MAXA + MAXP) * W], fp8, tag="l8")
                nc.sync.dma_start(
                    out=t8[:, : fk * W],
                    in_=pred8[:, s8_0 : s8_0 + fk, :].rearrange("h s w -> h (s w)"),
                )
                if r == 0:
                    # small gaussian loads slot in after the first round's
                    # pred DMAs so the main stream starts immediately
                    nc.sync.dma_start(out=gy8_t[:], in_=gy8[:, :])
                    nc.sync.dma_start(out=gy16_t[:], in_=gy16[:, :])
                    nc.sync.dma_start(out=gx_t[:], in_=gx[:, :])

                # ACT: sum of squares over its fp8 share -> outcols[r]
                nc.scalar.activation(
                    out=actout[:, : ak * W],
                    in_=t8[:, : ak * W],
                    func=mybir.ActivationFunctionType.Square,
                    accum_out=outcols[:, r : r + 1],
                )

                # DVE: square bf16 share at 2x, halving tree, short reduce
                n = dk * W
                nc.vector.tensor_mul(sq16[:, :n], t16[:, :n], t16[:, :n])
                while n > 256:
                    h = n // 2
                    nc.vector.tensor_add(sq16[:, :h], sq16[:, :h], sq16[:, h:n])
                    n = h
                nc.vector.reduce_sum(
                    outcols[:, NR + r : NR + r + 1], sq16[:, :n],
                    axis=mybir.AxisListType.X,
                )

                # Pool: square fp8 share, halve to 256, add into poolacc
                n = pk * W
                nc.gpsimd.tensor_mul(
                    sq8[:, :n], t8[:, ak * W : fk * W], t8[:, ak * W : fk * W]
                )
                while n > 256:
                    h = n // 2
                    nc.gpsimd.tensor_add(sq8[:, :h], sq8[:, :h], sq8[:, h:n])
                    n = h
                nc.gpsimd.tensor_add(
                    poolacc[:, :n], poolacc[:, :n], sq8[:, :n]
                )

                # cross term: m'_s = pred_s^T @ gy_s per slice -> psum column
                ps, col0 = (psA, g0) if r < NR - 1 else (psB, g0 - n_main)
                for i in range(fk):
                    nc.tensor.matmul(
                        ps[:, col0 + i : col0 + i + 1],
                        t8[:, i * W : (i + 1) * W],
                        gy8_t[:, s8_0 + i : s8_0 + i + 1],
                        start=True,
                        stop=True,
                    )
                for j in range(dk):
                    nc.tensor.matmul(
                        ps[:, col0 + fk + j : col0 + fk + j + 1],
                        t16[:, j * W : (j + 1) * W],
                        gy16_t[:, s16_0 + j : s16_0 + j + 1],
                        start=True,
                        stop=True,
                    )

                if r == NR - 2:
                    # grouped prod over all main-round m' columns; runs
                    # while the last round still streams
                    prodA = work.tile([128, n_main], f32, tag="prodA")
                    nc.vector.tensor_mul(prodA[:], psA[:], gx_t[:, :n_main])
                    nc.vector.reduce_sum(
                        outcols[:, 2 * NR + 1 : 2 * NR + 2], prodA[:],
                        axis=mybir.AxisListType.X,
                    )

                s8_0 += fk
                s16_0 += dk
                g0 += ak + dk + pk

            # tail: pool accumulator reduce, last-round prod
            nc.vector.reduce_sum(
                outcols[:, 2 * NR : 2 * NR + 1], poolacc[:],
                axis=mybir.AxisListType.X,
            )
            prodB = work.tile([128, n_last], f32, tag="prodB")
            nc.vector.tensor_mul(prodB[:], psB[:], gx_t[:, n_main:])
            nc.vector.reduce_sum(
                outcols[:, 2 * NR + 2 : 2 * NR + 3], prodB[:],
                axis=mybir.AxisListType.X,
            )

            nc.sync.dma_start(out=partials[:, :], in_=outcols[:])

    nc.finalize()
    return nc


def _gaussians(proj_mats_batch, joints_3d_gt_batch):
    """1D gaussians gy [B,V,J,H], gx [B,V,J,W] in float32 (reference math)."""
    joints = joints_3d_gt_batch.astype(np.float32)
    ones = np.ones(joints.shape[:-1] + (1,), dtype=np.float32)
    joints_h = np.concatenate([joints, ones], axis=-1)  # [B, J, 4]
    proj = np.einsum(
        "bvcd,bjd->bvjc", proj_mats_batch.astype(np.float32), joints_h
    ).astype(np.float32)  # [B, V, J, 3]
    joints_2d = proj[..., :2] / proj[..., 2:3]  # (x, y)
    xs = np.arange(W, dtype=np.float32)
    ys = np.arange(H, dtype=np.float32)
    dx2 = (xs - joints_2d[..., 0, None]) ** 2  # [B,V,J,W]
    dy2 = (ys - joints_2d[..., 1, None]) ** 2  # [B,V,J,H]
    gx = np.exp(-0.5 * dx2).astype(np.float32)
    gy = np.exp(-0.5 * dy2).astype(np.float32)
    return gy, gx


def _split_masks():
    """Boolean masks over the 272 per-core slices: fp8 vs bf16 stream."""
    m8 = np.zeros(SLICES, dtype=bool)
    g0 = 0
    for ak, dk, pk in ROUNDS:
        m8[g0 : g0 + ak + pk] = True       # fp8: ACT share then Pool share
        g0 += ak + pk + dk                 # bf16 share follows
    return m8, ~m8


def kernel(heatmaps_pred, proj_mats_batch, joints_3d_gt_batch, joints_3d_valid_batch,
           _profile=None):
    heatmaps_pred = np.asarray(heatmaps_pred, dtype=np.float32)
    gy, gx = _gaussians(np.asarray(proj_mats_batch), np.asarray(joints_3d_gt_batch))

    # s3 = sum over slices of (sum_h gy^2) * (sum_w gx^2), exact in f64
    s3 = float(
        ((gy.astype(np.float64) ** 2).sum(-1) * (gx.astype(np.float64) ** 2).sum(-1)).sum()
    )

    if "nc" not in _CACHE:
        _CACHE["nc"] = _build_nc()
    nc = _CACHE["nc"]

    m8, m16 = _split_masks()
    in_maps = []
    for c in range(N_CORES):
        bsl = slice(B_LOC * c, B_LOC * (c + 1))
        # slice order: (b_local, v, j) -> s ; pred h-major [H, n, W]
        pred_c = heatmaps_pred[bsl].reshape(SLICES, H, W)
        pred8 = np.ascontiguousarray(
            pred_c[m8].transpose(1, 0, 2).astype(ml_dtypes.float8_e4m3)
        )
        pred16 = np.ascontiguousarray(
            pred_c[m16].transpose(1, 0, 2).astype(ml_dtypes.bfloat16)
        )
        gy_c = gy[bsl].reshape(SLICES, H)
        gy8 = np.ascontiguousarray(gy_c[m8].T.astype(ml_dtypes.float8_e4m3))
        gy16 = np.ascontiguousarray(gy_c[m16].T.astype(ml_dtypes.bfloat16))
        gx_c = np.ascontiguousarray(
            gx[bsl].reshape(SLICES, W).T.astype(ml_dtypes.bfloat16)
        )
        in_maps.append(
            {"pred8": pred8, "pred16": pred16, "gy8": gy8, "gy16": gy16,
             "gx": gx_c}
        )

    res = run_bass_kernel_spmd(nc, in_maps, core_ids=list(range(N_CORES)))
    if _profile is not None:
        _profile["result"] = res
        _profile["in_maps"] = in_maps

    s1 = 0.0
    s2 = 0.0
    for c in range(N_CORES):
        p = res.results[c]["partials"].astype(np.float64)
        s1 += p[:, : 2 * NR + 1].sum()
        s2 += p[:, 2 * NR + 1 :].sum()

    total = s1 - 2.0 * s2 + s3
    return np.float32(total / (B * V * J * H * W))
